# revision 1
# baseline (speedup 1.0000x reference)
"""Causal self-attention on 8 trn2 cores.

Sharding: core c = 2*b + g handles batch b (of 4) and head group g (of 2,
8 heads each).  Tensor-parallel over heads for qkv/proj; host sums the two
w_proj partials per batch.

Layout per core (T=2048, C=1024, D=64, HG=8 local heads):
  - qkT  [1024, 2048]  : rows = [q heads (8*64) | k heads (8*64)], cols = t.
    Computed as w_qk.T @ x.T so Q^T/K^T come out directly.
  - v    [2048, 8, 65] : v natural layout + ones column per head (65th) so
    the PV matmul's 65th output row accumulates the softmax denominator.
  - attention in [k, q] orientation: S^T tile = kT_slice.T @ qT, exp via
    ACT (scale=1/8 folded in), causal mask by multiplying precomputed 0/1
    tiles on the 128-wide diagonal blocks, fully-masked blocks skipped.
  - yT [heads*64, 2048] accumulated in PSUM [65, 2048] per head; denom row
    reciprocal bounced through DRAM for a partition-broadcast DMA, then one
    DVE multiply normalizes.
  - proj: out[t, n] = yT.T @ w_p rows for this head group (partial).

All matmul operands are float32r (full-rate PE, ~1.5e-4 rel err).
"""

import numpy as np

import concourse.bacc as bacc
import concourse.bass as bass
import concourse.tile as tile
import concourse.mybir as mybir
from concourse.bass_utils import run_bass_kernel_spmd

F32 = mybir.dt.float32
F32R = mybir.dt.float32r
AF = mybir.ActivationFunctionType

B, T, C, H = 4, 2048, 1024, 16
D = C // H  # 64
HG = 8  # heads per core
NCB = C // 128  # 8 contraction chunks
NTB = T // 512  # 4 t blocks
NKT = T // 128  # 16 k tiles
SCALE = 1.0 / 8.0  # 1/sqrt(D)

_cache = {}
import os
KPHASES = int(os.environ.get("KPHASES", "3"))
KTT = int(os.environ.get("KTT", "16"))
KPROJ = int(os.environ.get("KPROJ", "7"))  # 1=mm 2=copy 4=dma


def build_nc():
    if "nc" in _cache:
        return _cache["nc"]
    nc = bacc.Bacc("TRN2", target_bir_lowering=False, debug=False, num_devices=8)

    xt_d = nc.dram_tensor("xt", [C, T], F32, kind="ExternalInput").ap()
    wqk_d = nc.dram_tensor("wqk", [C, 2 * HG * D], F32, kind="ExternalInput").ap()
    wv_d = nc.dram_tensor("wv", [C, HG * D], F32, kind="ExternalInput").ap()
    wp_d = nc.dram_tensor("wp", [HG * D, C], F32, kind="ExternalInput").ap()
    masks_d = nc.dram_tensor("masks", [128, 128], F32, kind="ExternalInput").ap()
    ones_d = nc.dram_tensor("ones", [128, HG], F32, kind="ExternalInput").ap()
    onesb_d = nc.dram_tensor("onesb", [1, 64], F32, kind="ExternalInput").ap()
    out_d = nc.dram_tensor("out", [T, C], F32, kind="ExternalOutput").ap()

    with tile.TileContext(nc) as tc:
        with (
            tc.tile_pool(name="persist", bufs=1) as persist,
            tc.tile_pool(name="qv", bufs=1) as qv,
            tc.tile_pool(name="dram", bufs=1, space="DRAM") as dramp,
        ):
            masks_sb = persist.tile([128, 128], F32, tag="masks", name="masks_sb")
            nc.sync.dma_start(masks_sb, masks_d)

            qkT = [qv.tile([128, T], F32R, tag=f"qkT{j}", name=f"qkT{j}") for j in range(8)]
            v_sb = [qv.tile([128, HG, 65], F32R, tag=f"v{t}", name=f"v{t}") for t in range(NKT)]
            ones_src = ones_d.rearrange("p (h o) -> p h o", o=1).bitcast(F32R)
            for tt in range(NKT):
                nc.sync.dma_start(v_sb[tt][:, :, 64:65], ones_src)

            # ---- phase 1: qkv projections ----
            with (
                tc.tile_pool(name="w1", bufs=1) as w1,
                tc.tile_pool(name="xt", bufs=16) as xtp,
                tc.tile_pool(name="ps1", bufs=4, space="PSUM") as ps1,
            ):
                wqk_sb = [w1.tile([128, 2 * HG * D], F32R, tag=f"wqk{c}", name=f"wqk{c}") for c in range(NCB)]
                wv_sb = [w1.tile([128, HG * D], F32R, tag=f"wv{c}", name=f"wv{c}") for c in range(NCB)]
                for cb in range(NCB):
                    nc.scalar.dma_start(wqk_sb[cb], wqk_d[128 * cb : 128 * (cb + 1), :].bitcast(F32R))
                    nc.scalar.dma_start(wv_sb[cb], wv_d[128 * cb : 128 * (cb + 1), :].bitcast(F32R))

                for tb in range(NTB):
                    xts = []
                    for cb in range(NCB):
                        xt_t = xtp.tile([128, 512], F32R, tag="xt", name="xt_t")
                        nc.sync.dma_start(
                            xt_t,
                            xt_d[128 * cb : 128 * (cb + 1), 512 * tb : 512 * (tb + 1)].bitcast(F32R),
                        )
                        xts.append(xt_t)
                    for jt in range(8):
                        ps = ps1.tile([128, 512], F32, tag="ps1", name="ps")
                        for cb in range(NCB):
                            nc.tensor.matmul(
                                ps,
                                wqk_sb[cb][:, 128 * jt : 128 * (jt + 1)],
                                xts[cb],
                                start=(cb == 0),
                                stop=(cb == NCB - 1),
                            )
                        nc.vector.tensor_copy(qkT[jt][:, 512 * tb : 512 * (tb + 1)], ps)
                    for t4 in range(4):
                        tt = 4 * tb + t4
                        ps = ps1.tile([128, 512], F32, tag="ps1", name="ps")
                        for cb in range(NCB):
                            nc.tensor.matmul(
                                ps,
                                xts[cb][:, 128 * t4 : 128 * (t4 + 1)],
                                wv_sb[cb],
                                start=(cb == 0),
                                stop=(cb == NCB - 1),
                            )
                        nc.vector.tensor_copy(
                            v_sb[tt][:, :, 0:64],
                            ps[:].rearrange("p (h e) -> p h e", h=HG),
                        )

            # ---- phase 2: attention per head ----
            with tc.tile_pool(name="yraw", bufs=1) as yrawp:
              with (
                  tc.tile_pool(name="strip", bufs=2, space="PSUM") as stripp,
                  tc.tile_pool(name="acc", bufs=1, space="PSUM") as accp,
                  tc.tile_pool(name="pp", bufs=2) as pp,
                  tc.tile_pool(name="rec", bufs=2) as recp,
              ):
                  yraw = [yrawp.tile([128, T], F32R, tag=f"yraw{j}", name=f"yraw{j}") for j in range(4 if KPHASES >= 2 else 0)]
                  onesb = persist.tile([1, 64], F32R, name="onesb")
                  nc.sync.dma_start(onesb, onesb_d[:].bitcast(F32R))
                  for h in range(HG if KPHASES >= 2 else 0):
                      jtq, rb = divmod(h, 2)
                      rb *= 64
                      jtk = 4 + jtq
                      acc = accp.tile([65, T], F32, tag="acc", name="acc")
                      for kt in range(NKT):
                          jq0, m = divmod(kt, 4)
                          off = 128 * m
                          qs = 512 * jq0 + off
                          lhsT_k = qkT[jtk][rb : rb + 64, 128 * kt : 128 * (kt + 1)]
                          for sbi in (0, 1):
                              lo = max(qs, 1024 * sbi)
                              hi = 1024 * (sbi + 1)
                              if lo >= hi:
                                  continue
                              strip = stripp.tile([128, 1024], F32, tag="strip", name="strip")
                              P = pp.tile([128, 1024], F32R, tag="P", name="P")
                              jqs = range(lo // 512, 2 * (sbi + 1))
                              for jq in jqs:
                                  a = max(lo, 512 * jq)
                                  b = 512 * (jq + 1)
                                  nc.tensor.matmul(
                                      strip[:, a - 1024 * sbi : b - 1024 * sbi],
                                      lhsT_k,
                                      qkT[jtq][rb : rb + 64, a:b],
                                      start=True,
                                      stop=True,
                                  )
                              loc = lo - 1024 * sbi
                              nc.scalar.activation(
                                  P[:, loc:1024], strip[:, loc:1024], AF.Exp, scale=SCALE
                              )
                              if 1024 * sbi <= 512 * jq0 < 1024 * (sbi + 1):
                                  dl = qs - 1024 * sbi
                                  nc.vector.tensor_mul(
                                      P[:, dl : dl + 128],
                                      P[:, dl : dl + 128],
                                      masks_sb,
                                  )
                              for jq in jqs:
                                  a = max(lo, 512 * jq)
                                  b = 512 * (jq + 1)
                                  nc.tensor.matmul(
                                      acc[:, a:b],
                                      v_sb[kt][:, h, :],
                                      P[:, a - 1024 * sbi : b - 1024 * sbi],
                                      start=(kt == 0),
                                      stop=(kt == 4 * jq + 3),
                                  )
                      rec = recp.tile([1, T], F32R, tag="rec", name="rec")
                      with nc.allow_low_precision(reason="softmax denom reciprocal as f32r matmul input"):
                          nc.vector.reciprocal(rec, acc[64:65, :])
                      nc.vector.tensor_copy(yraw[jtq][rb : rb + 64, :], acc[0:64, :])
                      bc = accp.tile([64, T], F32, tag="acc", name="bc")
                      for i in range(4):
                          nc.tensor.matmul(
                              bc[:, 512 * i : 512 * (i + 1)],
                              onesb,
                              rec[0:1, 512 * i : 512 * (i + 1)],
                              start=True,
                              stop=True,
                          )
                      nc.vector.tensor_mul(
                          yraw[jtq][rb : rb + 64, :],
                          yraw[jtq][rb : rb + 64, :],
                          bc,
                      )

              # ---- phase 3: output projection (partial; host sums pairs) ----
              with (
                  tc.tile_pool(name="w3", bufs=1) as w3,
                  tc.tile_pool(name="outp", bufs=3) as outp,
                  tc.tile_pool(name="ps3", bufs=4, space="PSUM") as ps3,
              ):
                  wp_sb = [w3.tile([128, C], F32R, tag=f"wp{j}", name=f"wp{j}") for j in range(4 if KPHASES >= 3 else 0)]
                  for jc in range(4 if KPHASES >= 3 else 0):
                      nc.scalar.dma_start(wp_sb[jc], wp_d[128 * jc : 128 * (jc + 1), :].bitcast(F32R))
                  for tt in range((KTT if KPHASES >= 3 else 0)):
                      ot = outp.tile([128, C], F32, tag="ot", name="ot")
                      for nb in (0, 1):
                          ps = ps3.tile([128, 512], F32, tag="ps3", name="ps")
                          for jc in range(4 if KPROJ & 1 else 0):
                              nc.tensor.matmul(
                                  ps,
                                  yraw[jc][:, 128 * tt : 128 * (tt + 1)],
                                  wp_sb[jc][:, 512 * nb : 512 * (nb + 1)],
                                  start=(jc == 0),
                                  stop=(jc == 3),
                              )
                          if KPROJ & 2:
                              nc.vector.tensor_copy(ot[:, 512 * nb : 512 * (nb + 1)], ps)
                          else:
                              nc.vector.memset(ot[:, 512 * nb : 512 * (nb + 1)], 0.0)
                      if KPROJ & 4:
                          nc.sync.dma_start(out_d[128 * tt : 128 * (tt + 1), :], ot)

    nc.compile()
    _cache["nc"] = nc
    return nc


def make_masks():
    i = np.arange(128)[:, None]
    j = np.arange(128)[None, :]
    return (j >= i).astype(np.float32)


def make_in_maps(x, w_qkv, w_proj):
    masks = make_masks()
    ones = np.ones((128, HG), np.float32)
    onesb = np.ones((1, 64), np.float32)
    wq, wk, wv = w_qkv[:, :C], w_qkv[:, C : 2 * C], w_qkv[:, 2 * C :]
    in_maps = []
    for c in range(8):
        b, g = divmod(c, 2)
        hs = slice(512 * g, 512 * (g + 1))
        in_maps.append(
            {
                "xt": np.ascontiguousarray(np.asarray(x[b]).T),
                "wqk": np.ascontiguousarray(
                    np.concatenate([wq[:, hs], wk[:, hs]], axis=1)
                ),
                "wv": np.ascontiguousarray(wv[:, hs]),
                "wp": np.ascontiguousarray(w_proj[512 * g : 512 * (g + 1), :]),
                "masks": masks,
                "ones": ones,
                "onesb": onesb,
            }
        )
    return in_maps


def kernel(x, w_qkv, w_proj):
    x = np.asarray(x, dtype=np.float32)
    w_qkv = np.asarray(w_qkv, dtype=np.float32)
    w_proj = np.asarray(w_proj, dtype=np.float32)
    nc = build_nc()
    in_maps = make_in_maps(x, w_qkv, w_proj)
    res = run_bass_kernel_spmd(nc, in_maps, core_ids=list(range(8)))
    out = np.empty((B, T, C), np.float32)
    for b in range(B):
        out[b] = res.results[2 * b]["out"] + res.results[2 * b + 1]["out"]
    return out

